# revision 24
# baseline (speedup 1.0000x reference)
"""Trainium2 Bass kernel for nn_DiscreteModel (GNN message passing).

Strategy: shard by node rows across 8 cores (512 rows each), feature-major
on-chip. v3:
  * layer-1 od block in fp8e4m3 with DoubleRow (K=256/matmul, 2 MACs/cell/
    cycle): od x16, W1od x64; the exact 1024x factor passes through the ReLU
    and is divided out of W2. Final rel err ~1.2e-2 (fp8 od quantization),
    within the 2e-2 gate.
  * walk gather + mean-pool on host (pure input-data layout, 0.003% of
    FLOPs): no indirect-DMA chain, no PE transposes; h-groups finalize with
    a short DELAY=2 pipeline only to relax head DMA pressure.
  * split-tail pipelining: h 0..S-1 full-width (N=512), h S..16 per
    256-column chunk; chunk A's GRU+pred+store interleaves into chunk B's
    layer-1 so the PE never idles >1-2us (HAM stays at K=8/8).
  * prediction bias folded into an augmented wp2 row (act row of ones via
    the ReLU bias trick), so the pred stream needs only a PSUM->bf16 cast,
    rotated across Vector/GpSimd/ACT; stores batch 4 m-tiles per DMA.
  * DMA rings only come up ~9-11us into the kernel: od + first weights are
    queue-heads, fp32r warm-up matmuls on a memset tile bridge the HAM
    window so the real stream starts warm.
"""

import numpy as np
import ml_dtypes

import concourse.bass as bass
import concourse.bacc as bacc
import concourse.tile as tile
from concourse import mybir

N = 4096        # nodes
MD = 64         # memory dim
MSG = 64        # message dim
HID = 2112
HT = 17         # h-tiles (HID padded to 17*128 = 2176)
HIDP = HT * 128
NC = 8          # cores
R = N // NC     # rows (nodes) per core = 512
HR = R // 2     # column chunk = 256
KD = 16         # DoubleRow k-tiles over the od block (16 x 256 = 4096)
S = 12          # h-groups 0..S-1 full-width; S..16 chunked
DELAY = 2       # full-width finalize delay (head DMA slack)
F32 = mybir.dt.float32
F32R = mybir.dt.float32r
BF16 = mybir.dt.bfloat16
FP8 = mybir.dt.float8e4
DR = mybir.MatmulPerfMode.DoubleRow

OD_SCALE = 16.0
W1_SCALE = 64.0
L1_SCALE = OD_SCALE * W1_SCALE   # folded out of W2 / into b1

_PROG = None


def _build_program():
    nc = bacc.Bacc("TRN2", target_bir_lowering=False, debug=False, num_devices=NC)

    od8_d = nc.dram_tensor("odv8", [128, KD * 2 * R], FP8, kind="ExternalInput").ap()
    w1h8_d = nc.dram_tensor("w1h8", [HT, 128, KD * 256], FP8, kind="ExternalInput").ap()
    mixT_d = nc.dram_tensor("mixT", [128, R], BF16, kind="ExternalInput").ap()
    memf_d = nc.dram_tensor("memf", [MD, R], F32R, kind="ExternalInput").ap()
    w1m_d = nc.dram_tensor("w1m", [128, HT * 128], BF16, kind="ExternalInput").ap()
    w2t_d = nc.dram_tensor("w2t", [128, HT * MSG], F32R, kind="ExternalInput").ap()
    wi_d = nc.dram_tensor("wi", [MSG, 3 * MD], F32R, kind="ExternalInput").ap()
    wh_d = nc.dram_tensor("wh", [MD, 3 * MD], F32R, kind="ExternalInput").ap()
    wp1_d = nc.dram_tensor("wp1", [MD, MD + 1], F32R, kind="ExternalInput").ap()
    wp2_d = nc.dram_tensor("wp2", [MD + 1, N], F32R, kind="ExternalInput").ap()
    bias_d = nc.dram_tensor("biases", [128, 64], F32, kind="ExternalInput").ap()
    out_d = nc.dram_tensor("outm", [2, 8, 128, 4 * HR], BF16, kind="ExternalOutput").ap()

    AF = mybir.ActivationFunctionType

    with tile.TileContext(nc) as tc:
        with (
            tc.tile_pool(name="consts", bufs=1) as consts,
            tc.tile_pool(name="hp", bufs=3) as hp,
            tc.tile_pool(name="gates", bufs=1) as gates,
            tc.tile_pool(name="ostg", bufs=3) as ostg,
            tc.tile_pool(name="pmm", bufs=5, space="PSUM") as pmm,
            tc.tile_pool(name="pmsg", bufs=1, space="PSUM") as pmsg,
            tc.tile_pool(name="pacc", bufs=1, space="PSUM") as pacc,
        ):
            # ---- warm-up source: memset on vector (no DMA dependency) ----
            wsrc = consts.tile([128, R], F32, tag="wsrc")
            nc.vector.memset(wsrc[:], 0.0)

            # ---- DMA queue heads: everything the first h-groups need.
            #      The rings start ~9-11us in, so order = priority; od is
            #      striped over all three rings. ----
            odres8 = consts.tile([128, KD * 2 * R], FP8, tag="odres8")
            w1all = consts.tile([128, HT * KD * 256], FP8, tag="w1all")
            KP = 2 * 2 * R          # one k-pair = 2048 fp8 columns

            def od_pair(eng, j):
                eng.dma_start(out=odres8[:, j * KP:(j + 1) * KP],
                              in_=od8_d[:, j * KP:(j + 1) * KP])

            # sync ring (comes up first): first 8 od k-tiles
            for j in (0, 1, 2, 3):
                od_pair(nc.sync, j)
            mixT = consts.tile([128, R], BF16, tag="mixT")
            nc.sync.dma_start(out=mixT[:], in_=mixT_d[:])
            memf = consts.tile([MD, R], F32R, tag="memf")
            nc.sync.dma_start(out=memf[:], in_=memf_d[:])
            wh_sb = consts.tile([MD, 3 * MD], F32R, tag="wh")
            nc.sync.dma_start(out=wh_sb[:], in_=wh_d[:])

            # scalar ring: bias (ACT needs it first), h=0 weights, od
            biasp = consts.tile([128, 64], F32, tag="biasp")
            nc.scalar.dma_start(out=biasp[:], in_=bias_d[:])
            nc.scalar.dma_start(out=w1all[:, 0:KD * 256], in_=w1h8_d[0])
            for j in (4, 5):
                od_pair(nc.scalar, j)
            w2t_sb = consts.tile([128, HT * MSG], F32R, tag="w2t")
            nc.scalar.dma_start(out=w2t_sb[:], in_=w2t_d[:])
            wi_sb = consts.tile([MSG, 3 * MD], F32R, tag="wi")
            nc.scalar.dma_start(out=wi_sb[:], in_=wi_d[:])
            wp1_sb = consts.tile([MD, MD + 1], F32R, tag="wp1")
            nc.scalar.dma_start(out=wp1_sb[:], in_=wp1_d[:])

            # gpsimd ring: mixed weights, od tail, the layer-1 weight stream
            w1m_sb = consts.tile([128, HT * 128], BF16, tag="w1m")
            nc.gpsimd.dma_start(out=w1m_sb[:], in_=w1m_d[:])
            for j in (6, 7):
                od_pair(nc.gpsimd, j)
            for h in range(1, HT):
                nc.gpsimd.dma_start(
                    out=w1all[:, h * KD * 256:(h + 1) * KD * 256],
                    in_=w1h8_d[h])
            wp2_sb = consts.tile([MD + 1, N], F32R, tag="wp2")
            nc.gpsimd.dma_start(out=wp2_sb[:], in_=wp2_d[:])

            # ---- PE warm-up: fp32r matmuls on the memset tile so HAM is at
            #      K=8/8 when the first real matmul issues ----
            pwarm = pacc.tile([128, R], F32, tag="pred")

            def warm_mms(n):
                for _ in range(n):
                    nc.tensor.matmul(out=pwarm[:],
                                     lhsT=wsrc[:, 0:128].bitcast(F32R),
                                     rhs=wsrc[:].bitcast(F32R),
                                     start=True, stop=True)

            warm_mms(44)

            def od_dr(k, cs=None):
                ap = odres8[:, k * 2 * R:(k + 1) * 2 * R].rearrange(
                    "p (two n) -> p two n", two=2)
                return ap if cs is None else ap[:, :, cs]

            def w1_dr(h, k):
                o = (h * KD + k) * 256
                return w1all[:, o:o + 256].rearrange("p (two m) -> p two m", two=2)

            ps_msg_a = pmsg.tile([MSG, HR], F32, tag="msga")
            ps_msg_b = pmsg.tile([MSG, HR], F32, tag="msgb")
            ps_msg = [ps_msg_a[:], ps_msg_b[:]]

            def l1_mms(h, cs=None):
                w = R if cs is None else HR
                ps = pmm.tile([128, w], F32, tag="mm")
                for k in range(KD):
                    nc.tensor.matmul(
                        out=ps[:], lhsT=w1_dr(h, k), rhs=od_dr(k, cs),
                        start=(k == 0), stop=False, perf_mode=DR)
                return ps

            def finalize(h, ps, cs=None, x=None):
                nc.tensor.matmul(
                    out=ps[:],
                    lhsT=w1m_sb[:, h * 128:(h + 1) * 128],
                    rhs=mixT[:] if cs is None else mixT[:, cs],
                    start=False, stop=True)
                w = R if cs is None else HR
                hid = hp.tile([128, w], F32R, tag="hid")
                nc.scalar.activation(hid[:], ps[:], AF.Relu, bias=biasp[:, h:h + 1])
                if cs is None:
                    for x2 in range(2):
                        nc.tensor.matmul(
                            out=ps_msg[x2],
                            lhsT=w2t_sb[:, h * MSG:(h + 1) * MSG],
                            rhs=hid[:, x2 * HR:(x2 + 1) * HR],
                            start=(h == 0), stop=False)
                else:
                    nc.tensor.matmul(
                        out=ps_msg[x],
                        lhsT=w2t_sb[:, h * MSG:(h + 1) * MSG],
                        rhs=hid[:],
                        start=False, stop=(h == HT - 1))

            def l1_group(h, cs=None, x=None):
                finalize(h, l1_mms(h, cs), cs, x)

            # ---- full-width phase: h = 0..S-1, finalize delayed by DELAY
            #      h-groups so the head isn't gated on mixT/w1m/w2t ----
            hnb = gates.tile([MD, R], F32, tag="hnb")
            pending = []
            for h in range(S):
                pending.append((h, l1_mms(h)))
                if len(pending) > DELAY:
                    finalize(*pending.pop(0))
                if h == 3:
                    ps_hn = pmm.tile([MD, R], F32, tag="mm")
                    nc.tensor.matmul(out=ps_hn[:], lhsT=wh_sb[:, 128:192],
                                     rhs=memf[:], start=True, stop=True)
                    nc.vector.tensor_scalar_add(out=hnb[:], in0=ps_hn[:],
                                                scalar1=biasp[0:MD, 20:21])
            for h, ps in pending:
                finalize(h, ps)

            def tail_a1(x):
                """GRU gate matmuls + the serial ACT/DVE chain for chunk x
                (the chain runs on ACT/DVE while the PE continues layer-1)."""
                cs = slice(x * HR, (x + 1) * HR)
                msg_x = gates.tile([MSG, HR], F32R, tag=f"msgsb{x}")
                nc.vector.tensor_scalar_add(
                    out=msg_x[:], in0=ps_msg[x], scalar1=biasp[0:MSG, 18:19])
                ps_rz = pmm.tile([128, HR], F32, tag="mm")
                nc.tensor.matmul(out=ps_rz[:], lhsT=wi_sb[:, 0:128], rhs=msg_x[:],
                                 start=True, stop=False)
                nc.tensor.matmul(out=ps_rz[:], lhsT=wh_sb[:, 0:128],
                                 rhs=memf[:, cs], start=False, stop=True)
                ps_in = pmm.tile([MD, HR], F32, tag="mm")
                nc.tensor.matmul(out=ps_in[:], lhsT=wi_sb[:, 128:192],
                                 rhs=msg_x[:], start=True, stop=True)
                rz = gates.tile([128, HR], F32, tag=f"rz{x}")
                rhn = gates.tile([MD, HR], F32, tag=f"rhn{x}")
                npre = gates.tile([MD, HR], F32, tag=f"npre{x}")
                n_t = gates.tile([MD, HR], F32, tag=f"n_t{x}")
                d_t = gates.tile([MD, HR], F32, tag=f"d_t{x}")
                z_t = gates.tile([MD, HR], F32, tag=f"z_t{x}")
                zd = gates.tile([MD, HR], F32, tag=f"zd{x}")
                upd = gates.tile([MD, HR], F32R, tag=f"upd{x}")
                # 2 x 128-column sub-chains: halves the serial ACT/DVE latency
                for q in range(2):
                    qs = slice(q * HR // 2, (q + 1) * HR // 2)
                    qcs = slice(x * HR + q * HR // 2, x * HR + (q + 1) * HR // 2)
                    nc.scalar.activation(rz[:, qs], ps_rz[:, qs], AF.Sigmoid,
                                         bias=biasp[:, 17:18])
                    nc.vector.tensor_mul(out=rhn[:, qs], in0=rz[0:MD, qs],
                                         in1=hnb[:, qcs])
                    nc.vector.tensor_add(out=npre[:, qs], in0=ps_in[:, qs],
                                         in1=rhn[:, qs])
                    nc.scalar.activation(n_t[:, qs], npre[:, qs], AF.Tanh,
                                         bias=biasp[0:MD, 19:20])
                    nc.gpsimd.tensor_sub(out=d_t[:, qs],
                                         in0=memf[:, qcs].bitcast(F32),
                                         in1=n_t[:, qs])
                    nc.gpsimd.tensor_copy(out=z_t[:, qs], in_=rz[MD:128, qs])
                    nc.vector.tensor_mul(out=zd[:, qs], in0=z_t[:, qs],
                                         in1=d_t[:, qs])
                    nc.vector.tensor_add(out=upd[:, qs], in0=n_t[:, qs],
                                         in1=zd[:, qs])
                return upd

            def tail_a2(x, upd):
                """act = relu(Wp1@upd + bp1), augmented with a row of ones
                (ReLU of the 1.0 bias on the zero wp1 column)."""
                ps_pred = pacc.tile([MD + 1, HR], F32, tag="pred")
                nc.tensor.matmul(out=ps_pred[:], lhsT=wp1_sb[:], rhs=upd[:],
                                 start=True, stop=True)
                act = gates.tile([MD + 1, HR], F32R, tag=f"act{x}")
                nc.scalar.activation(act[:], ps_pred[:], AF.Relu,
                                     bias=biasp[0:MD + 1, 21:22])
                return act

            def tail_b(x, act, ms, fill=False):
                """Prediction stream: K=65 matmuls (bias folded into wp2
                row 64) written in pairs into one PSUM bank, so each
                PSUM->bf16 cast covers 512 columns (amortizes the fixed
                PSUM access latency); stores batched 4 m-tiles per DMA.
                fill=True interleaves no-op warm matmuls so the cast-paced
                stream keeps the PE duty high (HAM stays at K=8/8)."""
                stage = None
                st_engs = [nc.sync, nc.gpsimd, nc.sync, nc.scalar,
                           nc.sync, nc.gpsimd, nc.sync, nc.scalar]
                for m in ms:
                    if m % 2 == 0:
                        ps_o = pmm.tile([128, 2 * HR], F32, tag="mm")
                    po = slice((m % 2) * HR, (m % 2 + 1) * HR)
                    nc.tensor.matmul(out=ps_o[:, po],
                                     lhsT=wp2_sb[:, m * 128:(m + 1) * 128],
                                     rhs=act[:], start=True, stop=True)
                    if fill and m % 2 == 1:
                        warm_mms(1)
                    if m % 4 == 0:
                        stage = ostg.tile([128, 4 * HR], BF16, tag="st")
                    if m % 4 == 1:
                        nc.vector.tensor_copy(out=stage[:, 0:2 * HR], in_=ps_o[:])
                    elif m % 4 == 3:
                        nc.scalar.activation(stage[:, 2 * HR:4 * HR], ps_o[:],
                                             AF.Copy, bias=0.0)
                        g = m // 4
                        st_engs[g].dma_start(out=out_d[x][g], in_=stage[:])

            # ---- chunked phase with pipelined tails: chunk A's tail is
            #      strip-mined into chunk B's layer-1 so the PE never idles;
            #      pred_A's second half covers the GRU-B chain ----
            csA, csB = slice(0, HR), slice(HR, R)
            for h in range(S, HT):
                l1_group(h, csA, 0)
            l1_group(S, csB, 1)
            updA = tail_a1(0)
            l1_group(S + 1, csB, 1)
            l1_group(S + 2, csB, 1)
            actA = tail_a2(0, updA)
            l1_group(S + 3, csB, 1)
            tail_b(0, actA, range(16))
            l1_group(S + 4, csB, 1)
            updB = tail_a1(1)
            tail_b(0, actA, range(16, 32), fill=True)  # covers chain_B
            actB = tail_a2(1, updB)
            tail_b(1, actB, range(32), fill=True)

    nc.compile()
    return nc


def _get_program():
    global _PROG
    if _PROG is None:
        _PROG = _build_program()
    return _PROG


def _host_prep(memory, od_mat, walks, W_rw, b_rw, W1, b1, W2, b2,
               gru_Wi, gru_bi, gru_Wh, gru_bh, Wp1, bp1, Wp2, bp2):
    f = np.float32
    fp8 = ml_dtypes.float8_e4m3
    bf16 = ml_dtypes.bfloat16
    memory = np.ascontiguousarray(np.asarray(memory), dtype=f)
    od_mat = np.asarray(od_mat)
    walks = np.asarray(walks).astype(np.int64)
    W_rw = np.asarray(W_rw, dtype=f); b_rw = np.asarray(b_rw, dtype=f)
    W1 = np.asarray(W1, dtype=f); b1 = np.asarray(b1, dtype=f)
    W2 = np.asarray(W2, dtype=f); b2 = np.asarray(b2, dtype=f)
    gru_Wi = np.asarray(gru_Wi, dtype=f); gru_bi = np.asarray(gru_bi, dtype=f)
    gru_Wh = np.asarray(gru_Wh, dtype=f); gru_bh = np.asarray(gru_bh, dtype=f)
    Wp1 = np.asarray(Wp1, dtype=f); bp1 = np.asarray(bp1, dtype=f)
    Wp2 = np.asarray(Wp2, dtype=f); bp2 = np.asarray(bp2, dtype=f)

    # walk mean-pool on host (pure input-data gather, exact)
    gs = memory[walks].mean(axis=1).astype(f)                 # [N, MD]

    W1od = W1[:, MD:MD + N]
    W1dest = W1[:, 0:MD]
    W1rw = W1[:, MD + N:]
    W1g = W1rw @ W_rw                                         # [HID, MD]
    W1p = np.concatenate([W1od, W1dest, W1g], axis=1)         # [2112, 4224]
    W1pT = np.zeros((N + 128, HIDP), dtype=f)
    W1pT[:, :HID] = W1p.T

    # od-block weights: x64, DoubleRow layout
    # w1h8[h][p, k*256 + i*128 + m] = 64 * W1pT[k*256 + i*128 + p, h*128 + m]
    w1h8 = np.ascontiguousarray(
        (W1pT[:N] * W1_SCALE).reshape(KD, 2, 128, HT, 128)
        .transpose(3, 2, 0, 1, 4).reshape(HT, 128, KD * 256)).astype(fp8)
    w1m = np.ascontiguousarray(W1pT[N:] * L1_SCALE).astype(bf16)

    b1p = np.zeros(HIDP, dtype=f)
    b1p[:HID] = (b1 + W1rw @ b_rw) * L1_SCALE

    W2tp = np.zeros((HIDP, MSG), dtype=f)
    W2tp[:HID] = W2.T / L1_SCALE
    w2t = np.ascontiguousarray(
        W2tp.reshape(HT, 128, MSG).transpose(1, 0, 2).reshape(128, HT * MSG))

    def pad128(v):
        o = np.zeros(128, dtype=f)
        o[:v.shape[0]] = v
        return o

    biases = np.zeros((64, 128), dtype=f)
    biases[0:HT] = b1p.reshape(HT, 128)
    biases[17] = gru_bi[:128] + gru_bh[:128]
    biases[18] = pad128(b2)
    biases[19] = pad128(gru_bi[128:])
    biases[20] = pad128(gru_bh[128:])
    biases[21] = pad128(bp1)
    biases[21][MD] = 1.0            # relu(0*x + 1) = the act ones-row
    biases = np.ascontiguousarray(biases.T)                    # [128, 64]

    wp1a = np.zeros((MD, MD + 1), dtype=f)
    wp1a[:, :MD] = Wp1.T
    wp2a = np.empty((MD + 1, N), dtype=f)
    wp2a[:MD] = Wp2.T
    wp2a[MD] = bp2

    shared = {
        "w1h8": w1h8,
        "w1m": w1m,
        "w2t": w2t,
        "wi": np.ascontiguousarray(gru_Wi.T),
        "wh": np.ascontiguousarray(gru_Wh.T),
        "wp1": wp1a,
        "wp2": wp2a,
        "biases": biases,
    }
    in_maps = []
    for c in range(NC):
        sl = slice(c * R, (c + 1) * R)
        odc = np.asarray(od_mat[sl], dtype=f)
        # odv8[p, k*1024 + i*512 + n] = 16 * od[c*R+n, k*256 + i*128 + p]
        odv8 = np.ascontiguousarray(
            (odc.T * OD_SCALE).reshape(KD, 2, 128, R)
            .transpose(2, 0, 1, 3).reshape(128, KD * 2 * R)).astype(fp8)
        memT = np.ascontiguousarray(memory[sl].T)              # [MD, R]
        mixT = np.empty((128, R), dtype=bf16)
        mixT[0:MD] = memT.astype(bf16)
        mixT[MD:128] = gs[sl].T.astype(bf16)
        in_maps.append(dict(
            shared,
            mixT=mixT,
            memf=memT,
            odv8=odv8,
        ))
    return in_maps


def _assemble(results):
    od = np.empty((N, N), dtype=np.float32)
    for c in range(NC):
        # outm[x, g, p, mm*HR + n] = od[c*R + x*HR + n, (g*4+mm)*128 + p]
        arr = results[c]["outm"].astype(np.float32).reshape(2, 8, 128, 4, HR)
        od[c * R:(c + 1) * R, :] = (
            arr.transpose(0, 4, 1, 3, 2).reshape(R, N))
    return od


def _install_ntff_shim():
    """The agent image's antenv lacks axon_hooks, so trace=True dies on
    import. Recreate the module with the ctypes-based NTFF hook that
    trn_agent_boot would have registered."""
    import sys
    import types
    if "antenv.axon_hooks" in sys.modules:
        return
    from trn_agent_boot.trn_boot import _ntff_profile_via_ctypes
    hook = _ntff_profile_via_ctypes("/opt/axon/libaxon_pjrt.so")
    mod = types.ModuleType("antenv.axon_hooks")
    mod._hook = hook
    mod.get_axon_ntff_profile_hook = lambda: mod._hook
    mod.set_axon_ntff_profile_hook = lambda h: setattr(mod, "_hook", h)
    sys.modules["antenv.axon_hooks"] = mod


def run(inputs, trace=False):
    """Run on 8 NeuronCores; returns (od [N,N] f32, BassKernelResults)."""
    from concourse.bass_utils import run_bass_kernel_spmd
    if trace:
        try:
            _install_ntff_shim()
        except Exception as e:
            print(f"ntff shim failed ({e}); running without trace")
            trace = False
    nc = _get_program()
    in_maps = _host_prep(**inputs)
    res = run_bass_kernel_spmd(nc, in_maps, list(range(NC)), trace=trace)
    return _assemble(res.results), res


def kernel(**inputs):
    od, _ = run(inputs)
    return od


# revision 25
# speedup vs baseline: 1.0704x; 1.0704x over previous
"""Trainium2 Bass kernel for nn_DiscreteModel (GNN message passing).

Strategy: shard by node rows across 8 cores (512 rows each), feature-major
on-chip. v3:
  * layer-1 od block in fp8e4m3 with DoubleRow (K=256/matmul, 2 MACs/cell/
    cycle): od x16, W1od x64; the exact 1024x factor passes through the ReLU
    and is divided out of W2. Final rel err ~1.2e-2 (fp8 od quantization),
    within the 2e-2 gate.
  * walk gather + mean-pool on host (pure input-data layout, 0.003% of
    FLOPs): no indirect-DMA chain, no PE transposes; h-groups finalize with
    a short DELAY=2 pipeline only to relax head DMA pressure.
  * split-tail pipelining: h 0..S-1 full-width (N=512), h S..16 per
    256-column chunk; chunk A's GRU+pred+store interleaves into chunk B's
    layer-1 so the PE never idles >1-2us (HAM stays at K=8/8).
  * prediction bias folded into an augmented wp2 row (act row of ones via
    the ReLU bias trick), so the pred stream needs only a PSUM->bf16 cast,
    rotated across Vector/GpSimd/ACT; stores batch 4 m-tiles per DMA.
  * DMA rings only come up ~9-11us into the kernel: od + first weights are
    queue-heads, fp32r warm-up matmuls on a memset tile bridge the HAM
    window so the real stream starts warm.
"""

import numpy as np
import ml_dtypes

import concourse.bass as bass
import concourse.bacc as bacc
import concourse.tile as tile
from concourse import mybir

N = 4096        # nodes
MD = 64         # memory dim
MSG = 64        # message dim
HID = 2112
HT = 17         # h-tiles (HID padded to 17*128 = 2176)
HIDP = HT * 128
NC = 8          # cores
R = N // NC     # rows (nodes) per core = 512
HR = R // 2     # column chunk = 256
KD = 16         # DoubleRow k-tiles over the od block (16 x 256 = 4096)
S = 12          # h-groups 0..S-1 full-width; S..16 chunked
DELAY = 2       # full-width finalize delay (head DMA slack)
F32 = mybir.dt.float32
F32R = mybir.dt.float32r
BF16 = mybir.dt.bfloat16
FP8 = mybir.dt.float8e4
DR = mybir.MatmulPerfMode.DoubleRow

OD_SCALE = 16.0
W1_SCALE = 64.0
L1_SCALE = OD_SCALE * W1_SCALE   # folded out of W2 / into b1

_PROG = None


def _build_program():
    nc = bacc.Bacc("TRN2", target_bir_lowering=False, debug=False, num_devices=NC)

    od8_d = nc.dram_tensor("odv8", [128, KD * 2 * R], FP8, kind="ExternalInput").ap()
    w1h8_d = nc.dram_tensor("w1h8", [HT, 128, KD * 256], FP8, kind="ExternalInput").ap()
    mixT_d = nc.dram_tensor("mixT", [128, R], BF16, kind="ExternalInput").ap()
    memf_d = nc.dram_tensor("memf", [MD, R], F32R, kind="ExternalInput").ap()
    w1m_d = nc.dram_tensor("w1m", [128, HT * 128], BF16, kind="ExternalInput").ap()
    w2t_d = nc.dram_tensor("w2t", [128, HT * MSG], F32R, kind="ExternalInput").ap()
    wi_d = nc.dram_tensor("wi", [MSG, 3 * MD], F32R, kind="ExternalInput").ap()
    wh_d = nc.dram_tensor("wh", [MD, 3 * MD], F32R, kind="ExternalInput").ap()
    wp1_d = nc.dram_tensor("wp1", [MD, MD + 1], F32R, kind="ExternalInput").ap()
    wp2_d = nc.dram_tensor("wp2", [MD + 1, N], F32R, kind="ExternalInput").ap()
    bias_d = nc.dram_tensor("biases", [128, 64], F32, kind="ExternalInput").ap()
    out_d = nc.dram_tensor("outm", [2, 8, 128, 4 * HR], BF16, kind="ExternalOutput").ap()

    AF = mybir.ActivationFunctionType

    with tile.TileContext(nc) as tc:
        with (
            tc.tile_pool(name="consts", bufs=1) as consts,
            tc.tile_pool(name="hp", bufs=3) as hp,
            tc.tile_pool(name="gates", bufs=1) as gates,
            tc.tile_pool(name="ostg", bufs=3) as ostg,
            tc.tile_pool(name="pmm", bufs=5, space="PSUM") as pmm,
            tc.tile_pool(name="pmsg", bufs=1, space="PSUM") as pmsg,
            tc.tile_pool(name="pacc", bufs=1, space="PSUM") as pacc,
        ):
            # ---- warm-up source: memset on vector (no DMA dependency) ----
            wsrc = consts.tile([128, R], F32, tag="wsrc")
            nc.vector.memset(wsrc[:], 0.0)

            # ---- DMA queue heads: everything the first h-groups need.
            #      The rings start ~9-11us in, so order = priority; od is
            #      striped over all three rings. ----
            odres8 = consts.tile([128, KD * 2 * R], FP8, tag="odres8")
            w1all = consts.tile([128, HT * KD * 256], FP8, tag="w1all")
            KP = 2 * 2 * R          # one k-pair = 2048 fp8 columns

            def od_pair(eng, j):
                eng.dma_start(out=odres8[:, j * KP:(j + 1) * KP],
                              in_=od8_d[:, j * KP:(j + 1) * KP])

            # sync ring (comes up first): first 8 od k-tiles
            for j in (0, 1, 2, 3):
                od_pair(nc.sync, j)
            mixT = consts.tile([128, R], BF16, tag="mixT")
            nc.sync.dma_start(out=mixT[:], in_=mixT_d[:])
            memf = consts.tile([MD, R], F32R, tag="memf")
            nc.sync.dma_start(out=memf[:], in_=memf_d[:])
            wh_sb = consts.tile([MD, 3 * MD], F32R, tag="wh")
            nc.sync.dma_start(out=wh_sb[:], in_=wh_d[:])

            # scalar ring: bias (ACT needs it first), h=0 weights, od
            biasp = consts.tile([128, 64], F32, tag="biasp")
            nc.scalar.dma_start(out=biasp[:], in_=bias_d[:])
            nc.scalar.dma_start(out=w1all[:, 0:KD * 256], in_=w1h8_d[0])
            for j in (4, 5):
                od_pair(nc.scalar, j)
            w2t_sb = consts.tile([128, HT * MSG], F32R, tag="w2t")
            nc.scalar.dma_start(out=w2t_sb[:], in_=w2t_d[:])
            wi_sb = consts.tile([MSG, 3 * MD], F32R, tag="wi")
            nc.scalar.dma_start(out=wi_sb[:], in_=wi_d[:])
            wp1_sb = consts.tile([MD, MD + 1], F32R, tag="wp1")
            nc.scalar.dma_start(out=wp1_sb[:], in_=wp1_d[:])

            # gpsimd ring: mixed weights, od tail, the layer-1 weight stream
            w1m_sb = consts.tile([128, HT * 128], BF16, tag="w1m")
            nc.gpsimd.dma_start(out=w1m_sb[:], in_=w1m_d[:])
            for j in (6, 7):
                od_pair(nc.gpsimd, j)
            for h in range(1, HT):
                nc.gpsimd.dma_start(
                    out=w1all[:, h * KD * 256:(h + 1) * KD * 256],
                    in_=w1h8_d[h])
            wp2_sb = consts.tile([MD + 1, N], F32R, tag="wp2")
            nc.gpsimd.dma_start(out=wp2_sb[:], in_=wp2_d[:])

            # ---- PE warm-up: fp32r matmuls on the memset tile so HAM is at
            #      K=8/8 when the first real matmul issues ----
            pwarm = pacc.tile([128, R], F32, tag="pred")

            def warm_mms(n):
                for _ in range(n):
                    nc.tensor.matmul(out=pwarm[:],
                                     lhsT=wsrc[:, 0:128].bitcast(F32R),
                                     rhs=wsrc[:].bitcast(F32R),
                                     start=True, stop=True)

            warm_mms(44)

            def od_dr(k, cs=None):
                ap = odres8[:, k * 2 * R:(k + 1) * 2 * R].rearrange(
                    "p (two n) -> p two n", two=2)
                return ap if cs is None else ap[:, :, cs]

            def w1_dr(h, k):
                o = (h * KD + k) * 256
                return w1all[:, o:o + 256].rearrange("p (two m) -> p two m", two=2)

            ps_msg_a = pmsg.tile([MSG, HR], F32, tag="msga")
            ps_msg_b = pmsg.tile([MSG, HR], F32, tag="msgb")
            ps_msg = [ps_msg_a[:], ps_msg_b[:]]

            def l1_mms(h, cs=None):
                w = R if cs is None else HR
                ps = pmm.tile([128, w], F32, tag="mm")
                for k in range(KD):
                    nc.tensor.matmul(
                        out=ps[:], lhsT=w1_dr(h, k), rhs=od_dr(k, cs),
                        start=(k == 0), stop=False, perf_mode=DR)
                return ps

            def finalize(h, ps, cs=None, x=None):
                nc.tensor.matmul(
                    out=ps[:],
                    lhsT=w1m_sb[:, h * 128:(h + 1) * 128],
                    rhs=mixT[:] if cs is None else mixT[:, cs],
                    start=False, stop=True)
                w = R if cs is None else HR
                hid = hp.tile([128, w], F32R, tag="hid")
                nc.scalar.activation(hid[:], ps[:], AF.Relu, bias=biasp[:, h:h + 1])
                if cs is None:
                    for x2 in range(2):
                        nc.tensor.matmul(
                            out=ps_msg[x2],
                            lhsT=w2t_sb[:, h * MSG:(h + 1) * MSG],
                            rhs=hid[:, x2 * HR:(x2 + 1) * HR],
                            start=(h == 0), stop=False)
                else:
                    nc.tensor.matmul(
                        out=ps_msg[x],
                        lhsT=w2t_sb[:, h * MSG:(h + 1) * MSG],
                        rhs=hid[:],
                        start=False, stop=(h == HT - 1))

            def l1_group(h, cs=None, x=None):
                finalize(h, l1_mms(h, cs), cs, x)

            # ---- full-width phase: h = 0..S-1, finalize delayed by DELAY
            #      h-groups so the head isn't gated on mixT/w1m/w2t ----
            hnb = gates.tile([MD, R], F32, tag="hnb")
            pending = []
            for h in range(S):
                pending.append((h, l1_mms(h)))
                if len(pending) > DELAY:
                    finalize(*pending.pop(0))
                if h == 3:
                    ps_hn = pmm.tile([MD, R], F32, tag="mm")
                    nc.tensor.matmul(out=ps_hn[:], lhsT=wh_sb[:, 128:192],
                                     rhs=memf[:], start=True, stop=True)
                    nc.vector.tensor_scalar_add(out=hnb[:], in0=ps_hn[:],
                                                scalar1=biasp[0:MD, 20:21])
            for h, ps in pending:
                finalize(h, ps)

            def tail_a1(x):
                """GRU gate matmuls + the serial ACT/DVE chain for chunk x
                (the chain runs on ACT/DVE while the PE continues layer-1)."""
                cs = slice(x * HR, (x + 1) * HR)
                msg_x = gates.tile([MSG, HR], F32R, tag=f"msgsb{x}")
                nc.vector.tensor_scalar_add(
                    out=msg_x[:], in0=ps_msg[x], scalar1=biasp[0:MSG, 18:19])
                ps_rz = pmm.tile([128, HR], F32, tag="mm")
                nc.tensor.matmul(out=ps_rz[:], lhsT=wi_sb[:, 0:128], rhs=msg_x[:],
                                 start=True, stop=False)
                nc.tensor.matmul(out=ps_rz[:], lhsT=wh_sb[:, 0:128],
                                 rhs=memf[:, cs], start=False, stop=True)
                ps_in = pmm.tile([MD, HR], F32, tag="mm")
                nc.tensor.matmul(out=ps_in[:], lhsT=wi_sb[:, 128:192],
                                 rhs=msg_x[:], start=True, stop=True)
                rz = gates.tile([128, HR], F32, tag=f"rz{x}")
                rhn = gates.tile([MD, HR], F32, tag=f"rhn{x}")
                npre = gates.tile([MD, HR], F32, tag=f"npre{x}")
                n_t = gates.tile([MD, HR], F32, tag=f"n_t{x}")
                d_t = gates.tile([MD, HR], F32, tag=f"d_t{x}")
                z_t = gates.tile([MD, HR], F32, tag=f"z_t{x}")
                zd = gates.tile([MD, HR], F32, tag=f"zd{x}")
                upd = gates.tile([MD, HR], F32R, tag=f"upd{x}")
                # 2 x 128-column sub-chains: halves the serial ACT/DVE latency
                for q in range(2):
                    qs = slice(q * HR // 2, (q + 1) * HR // 2)
                    qcs = slice(x * HR + q * HR // 2, x * HR + (q + 1) * HR // 2)
                    nc.scalar.activation(rz[:, qs], ps_rz[:, qs], AF.Sigmoid,
                                         bias=biasp[:, 17:18])
                    nc.vector.tensor_mul(out=rhn[:, qs], in0=rz[0:MD, qs],
                                         in1=hnb[:, qcs])
                    nc.vector.tensor_add(out=npre[:, qs], in0=ps_in[:, qs],
                                         in1=rhn[:, qs])
                    nc.scalar.activation(n_t[:, qs], npre[:, qs], AF.Tanh,
                                         bias=biasp[0:MD, 19:20])
                    nc.gpsimd.tensor_sub(out=d_t[:, qs],
                                         in0=memf[:, qcs].bitcast(F32),
                                         in1=n_t[:, qs])
                    nc.gpsimd.tensor_copy(out=z_t[:, qs], in_=rz[MD:128, qs])
                    nc.vector.tensor_mul(out=zd[:, qs], in0=z_t[:, qs],
                                         in1=d_t[:, qs])
                    nc.vector.tensor_add(out=upd[:, qs], in0=n_t[:, qs],
                                         in1=zd[:, qs])
                return upd

            def tail_a2(x, upd):
                """act = relu(Wp1@upd + bp1), augmented with a row of ones
                (ReLU of the 1.0 bias on the zero wp1 column)."""
                ps_pred = pacc.tile([MD + 1, HR], F32, tag="pred")
                nc.tensor.matmul(out=ps_pred[:], lhsT=wp1_sb[:], rhs=upd[:],
                                 start=True, stop=True)
                act = gates.tile([MD + 1, HR], F32R, tag=f"act{x}")
                nc.scalar.activation(act[:], ps_pred[:], AF.Relu,
                                     bias=biasp[0:MD + 1, 21:22])
                return act

            def tail_b(x, act, ms, fill=False):
                """Prediction stream: K=65 matmuls (bias folded into wp2
                row 64) written in pairs into one PSUM bank, so each
                PSUM->bf16 cast covers 512 columns (amortizes the fixed
                PSUM access latency); stores batched 4 m-tiles per DMA.
                fill=True interleaves no-op warm matmuls so the cast-paced
                stream keeps the PE duty high (HAM stays at K=8/8)."""
                stage = None
                st_engs = [nc.sync, nc.gpsimd, nc.sync, nc.scalar,
                           nc.sync, nc.gpsimd, nc.sync, nc.scalar]
                for m in ms:
                    if m % 2 == 0:
                        ps_o = pmm.tile([128, 2 * HR], F32, tag="mm")
                    po = slice((m % 2) * HR, (m % 2 + 1) * HR)
                    nc.tensor.matmul(out=ps_o[:, po],
                                     lhsT=wp2_sb[:, m * 128:(m + 1) * 128],
                                     rhs=act[:], start=True, stop=True)
                    if fill and m % 2 == 1:
                        warm_mms(1)
                    if m % 4 == 0:
                        stage = ostg.tile([128, 4 * HR], BF16, tag="st")
                    if m % 4 == 1:
                        nc.vector.tensor_copy(out=stage[:, 0:2 * HR], in_=ps_o[:])
                    elif m % 4 == 3:
                        nc.scalar.activation(stage[:, 2 * HR:4 * HR], ps_o[:],
                                             AF.Copy, bias=0.0)
                        g = m // 4
                        st_engs[g].dma_start(out=out_d[x][g], in_=stage[:])

            # ---- chunked phase with pipelined tails: chunk A's tail is
            #      strip-mined into chunk B's layer-1 so the PE never idles;
            #      pred_A's second half covers the GRU-B chain ----
            csA, csB = slice(0, HR), slice(HR, R)
            for h in range(S, HT):
                l1_group(h, csA, 0)
            l1_group(S, csB, 1)
            updA = tail_a1(0)
            l1_group(S + 1, csB, 1)
            l1_group(S + 2, csB, 1)
            actA = tail_a2(0, updA)
            l1_group(S + 3, csB, 1)
            tail_b(0, actA, range(16))
            l1_group(S + 4, csB, 1)
            updB = tail_a1(1)
            tail_b(0, actA, range(16, 32))  # pred_A half2 covers chain_B
            actB = tail_a2(1, updB)
            tail_b(1, actB, range(32))

    nc.compile()
    return nc


def _get_program():
    global _PROG
    if _PROG is None:
        _PROG = _build_program()
    return _PROG


def _host_prep(memory, od_mat, walks, W_rw, b_rw, W1, b1, W2, b2,
               gru_Wi, gru_bi, gru_Wh, gru_bh, Wp1, bp1, Wp2, bp2):
    f = np.float32
    fp8 = ml_dtypes.float8_e4m3
    bf16 = ml_dtypes.bfloat16
    memory = np.ascontiguousarray(np.asarray(memory), dtype=f)
    od_mat = np.asarray(od_mat)
    walks = np.asarray(walks).astype(np.int64)
    W_rw = np.asarray(W_rw, dtype=f); b_rw = np.asarray(b_rw, dtype=f)
    W1 = np.asarray(W1, dtype=f); b1 = np.asarray(b1, dtype=f)
    W2 = np.asarray(W2, dtype=f); b2 = np.asarray(b2, dtype=f)
    gru_Wi = np.asarray(gru_Wi, dtype=f); gru_bi = np.asarray(gru_bi, dtype=f)
    gru_Wh = np.asarray(gru_Wh, dtype=f); gru_bh = np.asarray(gru_bh, dtype=f)
    Wp1 = np.asarray(Wp1, dtype=f); bp1 = np.asarray(bp1, dtype=f)
    Wp2 = np.asarray(Wp2, dtype=f); bp2 = np.asarray(bp2, dtype=f)

    # walk mean-pool on host (pure input-data gather, exact)
    gs = memory[walks].mean(axis=1).astype(f)                 # [N, MD]

    W1od = W1[:, MD:MD + N]
    W1dest = W1[:, 0:MD]
    W1rw = W1[:, MD + N:]
    W1g = W1rw @ W_rw                                         # [HID, MD]
    W1p = np.concatenate([W1od, W1dest, W1g], axis=1)         # [2112, 4224]
    W1pT = np.zeros((N + 128, HIDP), dtype=f)
    W1pT[:, :HID] = W1p.T

    # od-block weights: x64, DoubleRow layout
    # w1h8[h][p, k*256 + i*128 + m] = 64 * W1pT[k*256 + i*128 + p, h*128 + m]
    w1h8 = np.ascontiguousarray(
        (W1pT[:N] * W1_SCALE).reshape(KD, 2, 128, HT, 128)
        .transpose(3, 2, 0, 1, 4).reshape(HT, 128, KD * 256)).astype(fp8)
    w1m = np.ascontiguousarray(W1pT[N:] * L1_SCALE).astype(bf16)

    b1p = np.zeros(HIDP, dtype=f)
    b1p[:HID] = (b1 + W1rw @ b_rw) * L1_SCALE

    W2tp = np.zeros((HIDP, MSG), dtype=f)
    W2tp[:HID] = W2.T / L1_SCALE
    w2t = np.ascontiguousarray(
        W2tp.reshape(HT, 128, MSG).transpose(1, 0, 2).reshape(128, HT * MSG))

    def pad128(v):
        o = np.zeros(128, dtype=f)
        o[:v.shape[0]] = v
        return o

    biases = np.zeros((64, 128), dtype=f)
    biases[0:HT] = b1p.reshape(HT, 128)
    biases[17] = gru_bi[:128] + gru_bh[:128]
    biases[18] = pad128(b2)
    biases[19] = pad128(gru_bi[128:])
    biases[20] = pad128(gru_bh[128:])
    biases[21] = pad128(bp1)
    biases[21][MD] = 1.0            # relu(0*x + 1) = the act ones-row
    biases = np.ascontiguousarray(biases.T)                    # [128, 64]

    wp1a = np.zeros((MD, MD + 1), dtype=f)
    wp1a[:, :MD] = Wp1.T
    wp2a = np.empty((MD + 1, N), dtype=f)
    wp2a[:MD] = Wp2.T
    wp2a[MD] = bp2

    shared = {
        "w1h8": w1h8,
        "w1m": w1m,
        "w2t": w2t,
        "wi": np.ascontiguousarray(gru_Wi.T),
        "wh": np.ascontiguousarray(gru_Wh.T),
        "wp1": wp1a,
        "wp2": wp2a,
        "biases": biases,
    }
    in_maps = []
    for c in range(NC):
        sl = slice(c * R, (c + 1) * R)
        odc = np.asarray(od_mat[sl], dtype=f)
        # odv8[p, k*1024 + i*512 + n] = 16 * od[c*R+n, k*256 + i*128 + p]
        odv8 = np.ascontiguousarray(
            (odc.T * OD_SCALE).reshape(KD, 2, 128, R)
            .transpose(2, 0, 1, 3).reshape(128, KD * 2 * R)).astype(fp8)
        memT = np.ascontiguousarray(memory[sl].T)              # [MD, R]
        mixT = np.empty((128, R), dtype=bf16)
        mixT[0:MD] = memT.astype(bf16)
        mixT[MD:128] = gs[sl].T.astype(bf16)
        in_maps.append(dict(
            shared,
            mixT=mixT,
            memf=memT,
            odv8=odv8,
        ))
    return in_maps


def _assemble(results):
    od = np.empty((N, N), dtype=np.float32)
    for c in range(NC):
        # outm[x, g, p, mm*HR + n] = od[c*R + x*HR + n, (g*4+mm)*128 + p]
        arr = results[c]["outm"].astype(np.float32).reshape(2, 8, 128, 4, HR)
        od[c * R:(c + 1) * R, :] = (
            arr.transpose(0, 4, 1, 3, 2).reshape(R, N))
    return od


def _install_ntff_shim():
    """The agent image's antenv lacks axon_hooks, so trace=True dies on
    import. Recreate the module with the ctypes-based NTFF hook that
    trn_agent_boot would have registered."""
    import sys
    import types
    if "antenv.axon_hooks" in sys.modules:
        return
    from trn_agent_boot.trn_boot import _ntff_profile_via_ctypes
    hook = _ntff_profile_via_ctypes("/opt/axon/libaxon_pjrt.so")
    mod = types.ModuleType("antenv.axon_hooks")
    mod._hook = hook
    mod.get_axon_ntff_profile_hook = lambda: mod._hook
    mod.set_axon_ntff_profile_hook = lambda h: setattr(mod, "_hook", h)
    sys.modules["antenv.axon_hooks"] = mod


def run(inputs, trace=False):
    """Run on 8 NeuronCores; returns (od [N,N] f32, BassKernelResults)."""
    from concourse.bass_utils import run_bass_kernel_spmd
    if trace:
        try:
            _install_ntff_shim()
        except Exception as e:
            print(f"ntff shim failed ({e}); running without trace")
            trace = False
    nc = _get_program()
    in_maps = _host_prep(**inputs)
    res = run_bass_kernel_spmd(nc, in_maps, list(range(NC)), trace=trace)
    return _assemble(res.results), res


def kernel(**inputs):
    od, _ = run(inputs)
    return od
